# revision 52
# baseline (speedup 1.0000x reference)
"""LSTMCell-variant Bass kernel for 8 Trainium2 NeuronCores.

Reference computation (B = H = O = 2048, fp32):
    z_g  = h @ W_hg + x @ W_xg + b_xg          (4 gates g in {f,g,i,o})
    gate = act(LayerNorm(z_g))                  (sigmoid/tanh/sigmoid/sigmoid)
    c_t  = f @ c_states + g @ i                 (matmul gating, not elementwise)
    h_t  = tanh(c_t) @ o
    y_t  = h_t @ W_y + b_y
    returns (c_t, h_t, y_t)

Strategy: data-parallel over batch rows (256 rows/core). Each core computes
its 4 gate row-shards; i and o are AllGathered (they are the RIGHT operands
of the gate matmuls, so every core needs them in full); f, g, tanh(c_t), h_t
are only needed as row shards, k-major (contraction on partitions) for use
as the stationary matmul operand — produced by XBAR DMA transposes
(dma_start_transpose), which run on the DMA engines fully overlapped with
matmul streaming, so the PE never idles through a transpose block (PE-idle
windows >~3.5us trip the HW activity monitor into a ~50%-throttle ramp).

Precision plan (tolerance is 2e-2 absmax-rel; measured 1.59e-2):
  - i/o gate projections run fp8e4m3 DoubleRow (2x PE throughput): their
    values are fp8-roundtripped into the AllGather payload anyway, so
    projection quantization adds little. Weights are host-scaled x256 into
    fp8's normal range; LayerNorm is exactly scale-invariant so nothing
    un-scales. f/g projections stay bf16: f@c_states transmits fp8 error
    into c_t at full scale, and g's tanh (slope 1 vs sigmoid's 0.25)
    amplifies it; both breach tolerance (measured, not guessed).
  - i and o cross the AllGather as fp8e4m3 of 2*(gate-0.5) = tanh(LN(z)/2):
    centering halves the quantization step; the exact mean contribution is
    re-added per-row after the matmul: g@i = g@(i-.5) + .5*rowsum(g),
    tanh(c)@o = tanh(c)@(o-.5) + .5*rowsum(tanh(c)).
  - f@c and h@W_y stay bf16 (fp8 for c_states or W_y measurably breaches
    2e-2: un-normalized matmul outputs see the full per-element fp8 error
    at their absmax entries, and h_t's heavy tails make it worse).
  - g@i and tanh(c)@o run fp8 DoubleRow: the stationary gate carries
    gate/2, the gathered side carries 2*(gate-0.5).

Host-side staging (free w.r.t. HW time): weights and activations are fed
pre-transposed / pre-cast, in a "pair" layout [128, KT*N] where the
contraction row kc*128+p lives at partition p, block kc — so one DMA
fetches a [128, 2, N] DoubleRow-ready strip pair.
"""

import os
from contextlib import ExitStack

import numpy as np

os.environ.setdefault("MYCRO_LOCAL_CACHE", "1")

try:
    import concourse.bass as bass  # noqa: F401
except ImportError:  # pragma: no cover
    import sys

    sys.path.insert(0, "/opt/trn_rl_repo")
    import concourse.bass as bass  # noqa: F401

import concourse.mybir as mybir
import concourse.tile as tile
from concourse import bacc
from concourse.bass_utils import run_bass_kernel_spmd
from concourse.masks import make_identity

B = 2048
H = 2048
OD = 2048
NCORES = 8
BS = B // NCORES  # 256 batch rows per core
NB = BS // 128  # 2 row-chunks of 128
KT = H // 128  # 16 contraction chunks
NP = KT // 2  # 8 contraction chunk-pairs
NSL = 4  # moving slices of 512 per full-width strip
EPS = 1e-5

F32 = mybir.dt.float32
BF16 = mybir.dt.bfloat16
FP8 = mybir.dt.float8e4
DR = mybir.MatmulPerfMode.DoubleRow
AX = mybir.AxisListType
OP = mybir.AluOpType
AF = mybir.ActivationFunctionType

_cache = {}


def _body(ctx: ExitStack, tc, I, Outs, apply_affine: bool):
    nc = tc.nc

    const = ctx.enter_context(tc.tile_pool(name="const", bufs=1))
    persist = ctx.enter_context(tc.tile_pool(name="persist", bufs=1))
    wmov = ctx.enter_context(tc.tile_pool(name="wmov", bufs=5))
    bxp = ctx.enter_context(tc.tile_pool(name="bxp", bufs=2))
    rows = ctx.enter_context(tc.tile_pool(name="rows", bufs=2))
    stats = ctx.enter_context(tc.tile_pool(name="stats", bufs=6))
    zps = ctx.enter_context(tc.tile_pool(name="zps", bufs=8, space="PSUM"))
    dram = ctx.enter_context(tc.tile_pool(name="dram", bufs=1, space="DRAM"))

    epsb = const.tile([128, 1], F32, tag="epsb", name="epsb")
    nc.gpsimd.memset(epsb[:], EPS)
    ident = const.tile([128, 128], F32, tag="ident", name="ident")
    make_identity(nc, ident[:])
    ident_b = const.tile([128, 128], BF16, tag="ident_b", name="ident_b")
    nc.vector.tensor_copy(ident_b[:], ident[:])

    # Persistent k-major activations: [128 partitions, KT chunks, col block].
    # Chunk kc holds rows kc*128:(kc+1)*128 of the transposed activation,
    # ready to slice as a stationary operand or as a DoubleRow pair.
    def kmajor(name, dt=BF16):
        return persist.tile([128, KT, BS], dt, tag=name, name=name)

    def kmajor_b(name, dt=BF16, tag=None):
        # per-row-chunk tiles (contiguous XBAR-transpose destinations)
        return [
            persist.tile([128, KT, 128], dt, tag=f"{tag or name}{b}", name=f"{name}{b}")
            for b in range(NB)
        ]

    hT8 = kmajor("hT8", dt=FP8)
    xT8 = kmajor("xT8", dt=FP8)
    hT = kmajor("hT")
    xT = kmajor("xT")
    fT_b = kmajor_b("fT")
    gT_b = kmajor_b("gT8", dt=FP8)
    tcT_b = kmajor_b("tcT8", dt=FP8)
    # fT is dead after stage-2a's matmuls — htT reuses its SBUF.
    htT_b = kmajor_b("htT", tag="fT")

    def mono(t):
        return lambda ksl, b: t[:, ksl, b * 128 : (b + 1) * 128]

    def perb(lst):
        return lambda ksl, b: lst[b][:, ksl, :]

    # DMA queue discipline: the sync queue is a SERIAL dispatcher and carries
    # ONLY the dependency-free weight/moving-strip stream — any DMA that
    # waits on compute (payload writes, XBAR transposes, output stores)
    # placed there blocks every later strip load (measured: an 18us PE stall
    # at the stage-1/2 boundary from exactly this).  XBARs go on the scalar
    # hwdge queue; everything else latency-tolerant goes on gpsimd.
    # Four DMAs per tensor so the first k-chunks land on parallel queues.
    def load_kmajor(eng, tdst, tsrc):
        for q in range(4):
            kq = slice(q * (KT // 4), (q + 1) * (KT // 4))
            eng.dma_start(
                tdst[:, kq, :],
                I[tsrc][:, q * (KT // 4) * BS : (q + 1) * (KT // 4) * BS].rearrange(
                    "p (k n) -> p k n", k=KT // 4
                ),
            )

    # fp8 copies split across the scalar and gpsimd queues: the i/o gates
    # need them immediately, and sync starts gate-i's strips in parallel.
    load_kmajor(nc.scalar, hT8, "hT8")
    load_kmajor(nc.gpsimd, xT8, "xT8")

    # DRAM bounce buffers for the i/o AllGathers (fp8, centered at 0).
    io_in = {g: dram.tile([BS, H], FP8, tag=f"io_in_{g}", name=f"io_in_{g}") for g in "io"}
    io_full = {g: dram.tile([B, H], FP8, tag=f"io_full_{g}", name=f"io_full_{g}", addr_space="Shared") for g in "io"}

    # 0.5*rowsum corrections, one column per row-chunk.
    rs_g = persist.tile([128, NB], F32, tag="rs_g", name="rs_g")
    rs_tc = persist.tile([128, NB], F32, tag="rs_tc", name="rs_tc")

    def layernorm_act(z_sb, func, gate_out, ga_sb, be_sb, half_tanh=False):
        """z_sb [128, H] -> gate_out = func(LN(z)) (affine optional).

        Sum and sum-of-squares are both taken on the raw z (var = E[z^2]-m^2)
        so the Vector reduce and Scalar square run concurrently, and the
        centering+scaling collapses into the activation's scale/bias:
        func(inv*z - m*inv).  half_tanh emits tanh(LN(z)/2) = 2*(sigmoid-0.5)
        directly (the fp8 AllGather payload).
        """
        s1 = stats.tile([128, 1], F32, tag="s1", name="s1")
        nc.vector.tensor_reduce(s1[:], z_sb[:], AX.X, OP.add)
        trash = rows.tile([128, H], BF16, tag="trash", name="trash")
        ssq = stats.tile([128, 1], F32, tag="ssq", name="ssq")
        nc.scalar.activation(trash[:], z_sb[:], AF.Square, accum_out=ssq[:])
        negm = stats.tile([128, 1], F32, tag="negm", name="negm")
        nc.vector.tensor_scalar_mul(negm[:], s1[:], -1.0 / H)
        var = stats.tile([128, 1], F32, tag="var", name="var")
        nc.vector.tensor_scalar_mul(var[:], ssq[:], 1.0 / H)
        m2 = stats.tile([128, 1], F32, tag="m2", name="m2")
        nc.vector.tensor_tensor(m2[:], negm[:], negm[:], OP.mult)
        nc.vector.tensor_tensor(var[:], var[:], m2[:], OP.subtract)
        std = stats.tile([128, 1], F32, tag="std", name="std")
        nc.scalar.activation(std[:], var[:], AF.Sqrt, bias=epsb[:])
        inv = stats.tile([128, 1], F32, tag="inv", name="inv")
        nc.vector.reciprocal(inv[:], std[:])
        if apply_affine:
            nc.vector.tensor_scalar(
                out=z_sb[:], in0=z_sb[:], scalar1=negm[:], scalar2=inv[:],
                op0=OP.add, op1=OP.mult,
            )
            nc.vector.tensor_tensor(z_sb[:], z_sb[:], ga_sb[:], OP.mult)
            nc.vector.tensor_tensor(z_sb[:], z_sb[:], be_sb[:], OP.add)
            if half_tanh:
                nc.scalar.activation(gate_out[:], z_sb[:], AF.Tanh, scale=0.5)
            else:
                nc.scalar.activation(gate_out[:], z_sb[:], func)
            return
        mb = stats.tile([128, 1], F32, tag="mb", name="mb")
        nc.vector.tensor_tensor(mb[:], negm[:], inv[:], OP.mult)
        if half_tanh:
            inv2 = stats.tile([128, 1], F32, tag="inv2", name="inv2")
            nc.vector.tensor_scalar_mul(inv2[:], inv[:], 0.5)
            mb2 = stats.tile([128, 1], F32, tag="mb2", name="mb2")
            nc.vector.tensor_scalar_mul(mb2[:], mb[:], 0.5)
            nc.scalar.activation(gate_out[:], z_sb[:], AF.Tanh, scale=inv2[:], bias=mb2[:])
        else:
            nc.scalar.activation(gate_out[:], z_sb[:], func, scale=inv[:], bias=mb[:])

    def transpose_rows(src_sb, dst, scale=None):
        """src_sb [128, H] (bf16) -> dst [128, KT, 128] k-major, via PE
        transposes + V/S psum evictions (alternating engines).  Used only
        for fT/gT at the stage-1/2 boundary: XBAR DMA-transposes would be
        serialized behind the in-flight AllGathers there (measured on HW),
        so the PE does these two.  The kc%8 warm matmul keeps the HW
        activity monitor from dropping into its 50%-throttle ramp
        (transpose-mode does not count as PE activity).
        """
        for kc in range(KT):
            tp = zps.tile([128, 128], BF16, tag="z", name="tp")
            nc.tensor.transpose(tp[:], src_sb[:, kc * 128 : (kc + 1) * 128], ident_b[:])
            if kc % 8 == 3:
                warm = zps.tile([128, 512], F32, tag="z", name="warm")
                nc.tensor.matmul(
                    warm[:], ident_b[:], src_sb[:, 0:512], start=True, stop=True
                )
            d = dst[:, kc, :]
            if kc % 2 == 0:
                if scale is None:
                    nc.vector.tensor_copy(d, tp[:])
                else:
                    nc.vector.tensor_scalar_mul(d, tp[:], scale)
            else:
                nc.scalar.activation(d, tp[:], AF.Copy, scale=scale or 1.0)

    def accumulate(psums, phases, on_complete=None, quarter_first=False):
        """psums[b][j] += sum over phases of stat(k,b).T @ strip.

        phases: (stat_fn, dram_src, dtype, use_dr, paired_src).
        paired_src=True: dram_src is in pair layout [128, KT*N] so one DMA
        yields a [128, 2, N] strip pair (2N contiguous bytes/partition).
        Otherwise dram_src is row-major [K, N] and the pair takes two DMAs.
        use_dr: fp8 DoubleRow — one matmul consumes the whole pair.
        on_complete(b, j): emitted right after psum (b,j)'s stop matmul, so
        evictions enter the queues while the remaining psums' matmuls still
        stream (spreads V/S work, starts the XBAR transposes of the next
        stage's stationary operand before this stage's last matmul).
        """
        for p, (stat, dram_src, mdt, use_dr, paired) in enumerate(phases):
            n = dram_src.shape[-1] // (KT if paired else 1)
            for k2 in range(NP):
                w = wmov.tile([128, 2, n], mdt, tag="wm", name="wm")
                if quarter_first and paired and p == 0 and k2 == 0:
                    # Quarter-DMAs on parallel queues to cut first-matmul latency.
                    for m in range(2):
                        for hh in range(2):
                            nc.sync.dma_start(
                                w[:, m, hh * (n // 2) : (hh + 1) * (n // 2)],
                                dram_src[
                                    :,
                                    (2 * k2 + m) * n
                                    + hh * (n // 2) : (2 * k2 + m) * n
                                    + (hh + 1) * (n // 2),
                                ],
                            )
                elif paired:
                    nc.sync.dma_start(
                        w[:],
                        dram_src[:, 2 * k2 * n : (2 * k2 + 2) * n].rearrange(
                            "p (k n) -> p k n", k=2
                        ),
                    )
                else:
                    for m in range(2):
                        nc.sync.dma_start(
                            w[:, m, :],
                            dram_src[(2 * k2 + m) * 128 : (2 * k2 + m + 1) * 128, :],
                        )
                start = p == 0 and k2 == 0
                stop = p == len(phases) - 1 and k2 == NP - 1
                # j-outer: psums complete j-major, so the next stage's first
                # stationary slices (j=0, both b) finish their evict+XBAR
                # chain first and its matmuls start with minimal bubble.
                if use_dr:
                    for j in range(NSL):
                        for b in range(NB):
                            nc.tensor.matmul(
                                psums[b][j][:],
                                stat(slice(2 * k2, 2 * k2 + 2), b),
                                w[:, :, j * 512 : (j + 1) * 512],
                                start=start,
                                stop=stop,
                                perf_mode=DR,
                            )
                            if stop and on_complete is not None:
                                on_complete(b, j)
                else:
                    for m in range(2):
                        for j in range(NSL):
                            for b in range(NB):
                                nc.tensor.matmul(
                                    psums[b][j][:],
                                    stat(2 * k2 + m, b),
                                    w[:, m, j * 512 : (j + 1) * 512],
                                    start=start and m == 0,
                                    stop=stop and m == 1,
                                )
                                if stop and m == 1 and on_complete is not None:
                                    on_complete(b, j)

    def new_psums():
        return [
            [zps.tile([128, 512], F32, tag="z", name="z") for _ in range(NSL)]
            for _ in range(NB)
        ]

    # ---- Stage 1: the four gates, in order i, o, f, g.
    # i/o first (fp8 DR; their AllGathers overlap the f/g bf16 gate work),
    # f before g so fT is ready when stage 2 opens with f@c_states, and g's
    # XBAR transpose + fp8 cast overlap stage-2a.
    gate_specs = [
        ("i", AF.Sigmoid),
        ("o", AF.Sigmoid),
        ("f", AF.Sigmoid),
        ("g", AF.Tanh),
    ]
    gate_sb = {"f": [], "g": []}
    for gname, func in gate_specs:
        z_sb = [rows.tile([128, H], F32, tag="z_sb", name="z_sb") for _ in range(NB)]
        psums = new_psums()
        if gname in ("i", "o"):
            accumulate(
                psums,
                [
                    (mono(hT8), I[f"W_h{gname}"], FP8, True, True),
                    (mono(xT8), I[f"W_x{gname}"], FP8, True, True),
                ],
                quarter_first=(gname == "i"),
            )
        else:
            accumulate(
                psums,
                [
                    (mono(hT), I[f"W_h{gname}"], BF16, False, True),
                    (mono(xT), I[f"W_x{gname}"], BF16, False, True),
                ],
            )
        if gname == "o":
            # bf16 activation copies for the f/g gates: on gpsimd so they
            # never contend with the strip stream; land long before f.
            load_kmajor(nc.gpsimd, hT, "hT")
            load_kmajor(nc.gpsimd, xT, "xT")
        # Bias (and affine) loads after the strips on sync: only needed at
        # eviction, and the strip stream runs far enough ahead.
        bx_sb = bxp.tile([128, H], BF16, tag="bx", name="bx")
        nc.sync.dma_start(bx_sb[:], I[f"bx_{gname}"][:])
        ga_sb = be_sb = None
        if apply_affine:
            ga_sb = bxp.tile([128, H], F32, tag="ga", name="ga")
            nc.sync.dma_start(ga_sb[:], I[f"ga_{gname}"][:])
            be_sb = bxp.tile([128, H], F32, tag="be", name="be")
            nc.sync.dma_start(be_sb[:], I[f"be_{gname}"][:])
        for b in range(NB):
            for j in range(NSL):
                col = slice(j * 512, (j + 1) * 512)
                nc.vector.tensor_tensor(
                    z_sb[b][:, col], psums[b][j][:], bx_sb[:, col], OP.add
                )
        for b in range(NB):
            if gname in ("i", "o"):
                # Emit the fp8 AllGather payload 2*(sigmoid(LN)-0.5) directly.
                # Payload write on gpsimd: in-order right before the gather
                # trigger (also on gpsimd), and off the strip stream.
                g8 = rows.tile([128, H], FP8, tag="g8", name="g8")
                layernorm_act(z_sb[b], func, g8, ga_sb, be_sb, half_tanh=True)
                nc.gpsimd.dma_start(io_in[gname][b * 128 : (b + 1) * 128, :], g8[:])
                continue
            gt = rows.tile([128, H], BF16, tag=f"gate_{gname}", name=f"gate_{gname}")
            layernorm_act(z_sb[b], func, gt, ga_sb, be_sb)
            gate_sb[gname].append(gt)
            if gname == "g":
                rs = stats.tile([128, 1], F32, tag="rs", name="rs")
                nc.vector.tensor_reduce(rs[:], gt[:], AX.X, OP.add)
                nc.vector.tensor_scalar_mul(rs_g[:, b : b + 1], rs[:], 0.5)
        if gname in ("i", "o"):
            nc.gpsimd.collective_compute(
                "AllGather",
                OP.bypass,
                replica_groups=[list(range(NCORES))],
                ins=[io_in[gname].opt()],
                outs=[io_full[gname].opt()],
            )

    # PE transposes for fT/gT, deferred here so they never block a gate's
    # matmul stream on a pending LayerNorm; f first (stage 2 opens with
    # f@c_states, and g's LN is still finishing while f's transposes run).
    for b in range(NB):
        transpose_rows(gate_sb["f"][b], fT_b[b][:])
    for b in range(NB):
        transpose_rows(gate_sb["g"][b], gT_b[b][:], scale=0.5)

    # ---- Stage 2: c_t = f @ c_states + g @ (i-.5) (+ .5*rowsum(g))
    # f@c_states first: pushes the g@i phase (and the i-AllGather deadline)
    # ~30us later.  Per-psum callbacks evict c, emit tanh, and XBAR-transpose
    # the tanh slice for stage 3 while the remaining DR matmuls stream.
    c_sb = [rows.tile([128, H], F32, tag="c_sb", name="c_sb") for _ in range(NB)]
    tc_sb = [rows.tile([128, H], BF16, tag="tc_sb", name="tc_sb") for _ in range(NB)]
    psums = new_psums()

    def s2_done(b, j):
        col = slice(j * 512, (j + 1) * 512)
        nc.vector.tensor_scalar(
            out=c_sb[b][:, col], in0=psums[b][j][:],
            scalar1=rs_g[:, b : b + 1], scalar2=None, op0=OP.add,
        )
        nc.scalar.activation(
            tc_sb[b][:, col], psums[b][j][:], AF.Tanh, bias=rs_g[:, b : b + 1]
        )

    accumulate(
        psums,
        [
            (perb(fT_b), I["c_states"], BF16, False, True),
            (perb(gT_b), io_full["i"], FP8, True, False),
        ],
        on_complete=s2_done,
    )
    # c output DMA + rowsum(tanh(c)) after the matmul stream (not latency-
    # critical; keeps the output DMAs from delaying stage-3's first strips).
    for b in range(NB):
        nc.gpsimd.dma_start(Outs["c_out"][b * 128 : (b + 1) * 128, :], c_sb[b][:])
        rs = stats.tile([128, 1], F32, tag="rs", name="rs")
        nc.vector.tensor_reduce(rs[:], tc_sb[b][:], AX.X, OP.add)
        nc.vector.tensor_scalar_mul(rs_tc[:, b : b + 1], rs[:], 0.5)
    # PE transposes for tcT (XBAR transposes here collide with collective
    # semaphore bookkeeping and each costs ~2us on a serial hwdge queue;
    # the PE does these faster and keeps its activity monitor warm).
    for b in range(NB):
        transpose_rows(tc_sb[b], tcT_b[b][:], scale=0.5)

    # ---- Stage 3: h_t = tanh(c_t)/2 @ 2*(o-.5) + .5*rowsum(tanh(c_t))
    h_sb = [rows.tile([128, H], F32, tag="h_sb", name="h_sb") for _ in range(NB)]
    hb_sb = [rows.tile([128, H], BF16, tag="hb_sb", name="hb_sb") for _ in range(NB)]
    psums = new_psums()

    def s3_done(b, j):
        col = slice(j * 512, (j + 1) * 512)
        nc.vector.tensor_scalar(
            out=h_sb[b][:, col], in0=psums[b][j][:],
            scalar1=rs_tc[:, b : b + 1], scalar2=None, op0=OP.add,
        )
        # bf16 twin (on Scalar) feeds the PE transpose for stage 4.
        nc.scalar.activation(
            hb_sb[b][:, col], psums[b][j][:], AF.Identity, bias=rs_tc[:, b : b + 1]
        )

    accumulate(psums, [(perb(tcT_b), io_full["o"], FP8, True, False)], on_complete=s3_done)
    for b in range(NB):
        nc.gpsimd.dma_start(Outs["h_out"][b * 128 : (b + 1) * 128, :], h_sb[b][:])
        transpose_rows(hb_sb[b], htT_b[b][:])

    # ---- Stage 4: y = h_t @ W_y + b_y  (bf16: fp8 W_y alone breaches 2e-2)
    by_sb = bxp.tile([128, OD], BF16, tag="bx", name="bx")
    nc.gpsimd.dma_start(by_sb[:], I["by_rep"][:])
    y_sb = [rows.tile([128, OD], F32, tag="z_sb", name="y_sb") for _ in range(NB)]
    psums = new_psums()

    def s4_done(b, j):
        col = slice(j * 512, (j + 1) * 512)
        nc.vector.tensor_tensor(
            y_sb[b][:, col], psums[b][j][:], by_sb[:, col], OP.add
        )
        # y goes out on sync: there is no later stage to block, and the
        # gpsimd software-DMA path costs ~6us of drain at the very tail.
        nc.sync.dma_start(
            Outs["y_out"][b * 128 : (b + 1) * 128, col], y_sb[b][:, col]
        )

    accumulate(psums, [(perb(htT_b), I["W_y"], BF16, False, True)], on_complete=s4_done)


def _build(apply_affine: bool):
    nc = bacc.Bacc(
        "TRN2",
        target_bir_lowering=False,
        debug=False,
        enable_asserts=False,
        num_devices=NCORES,
    )
    I = {}

    def di(name, shape, dt=F32):
        I[name] = nc.dram_tensor(name, list(shape), dt, kind="ExternalInput").ap()

    di("hT", (128, KT * BS), BF16)
    di("xT", (128, KT * BS), BF16)
    di("hT8", (128, KT * BS), FP8)
    di("xT8", (128, KT * BS), FP8)
    di("c_states", (128, KT * H), BF16)
    di("W_y", (128, KT * OD), BF16)
    di("by_rep", (128, OD), BF16)
    for g in "fgio":
        wdt = FP8 if g in "io" else BF16
        di(f"W_h{g}", (128, KT * H), wdt)
        di(f"W_x{g}", (128, KT * H), wdt)
        di(f"bx_{g}", (128, H), BF16)
        if apply_affine:
            di(f"ga_{g}", (128, H))
            di(f"be_{g}", (128, H))
    Outs = {
        n: nc.dram_tensor(n, [BS, H], F32, kind="ExternalOutput").ap()
        for n in ("c_out", "h_out", "y_out")
    }

    with tile.TileContext(nc) as tc, ExitStack() as ctx:
        _body(ctx, tc, I, Outs, apply_affine)
    nc.compile()
    return nc


def kernel(**inputs):
    inputs = {k: np.asarray(v, dtype=np.float32) for k, v in inputs.items()}
    apply_affine = not all(
        np.all(inputs[f"g_{g}"] == 1.0) and np.all(inputs[f"be_{g}"] == 0.0)
        for g in "fgio"
    )
    if apply_affine not in _cache:
        _cache[apply_affine] = _build(apply_affine)
    nc = _cache[apply_affine]

    import ml_dtypes

    bf16 = ml_dtypes.bfloat16
    f8 = ml_dtypes.float8_e4m3
    WS = np.float32(256.0)  # i/o weight pre-scale into fp8 normals (LN-invariant)

    def pair_layout(W, dt):
        # [K, N] -> [128, KT*N]: row kc*128+p lands at partition p, block kc.
        K, N = W.shape
        return np.ascontiguousarray(
            W.reshape(K // 128, 128, N).transpose(1, 0, 2).reshape(128, -1)
        ).astype(dt)

    def rep(v, dt=bf16):
        return np.ascontiguousarray(
            np.broadcast_to(v[None, :], (128, v.shape[0]))
        ).astype(dt)

    base = {
        "c_states": pair_layout(inputs["c_states"], bf16),
        "W_y": pair_layout(inputs["W_y"], bf16),
        "by_rep": rep(inputs["b_y"]),
    }
    for g in "fgio":
        if g in "io":
            base[f"W_h{g}"] = pair_layout(inputs[f"W_h{g}"] * WS, f8)
            base[f"W_x{g}"] = pair_layout(inputs[f"W_x{g}"] * WS, f8)
            base[f"bx_{g}"] = rep(inputs[f"b_x{g}"] * WS)
        else:
            base[f"W_h{g}"] = pair_layout(inputs[f"W_h{g}"], bf16)
            base[f"W_x{g}"] = pair_layout(inputs[f"W_x{g}"], bf16)
            base[f"bx_{g}"] = rep(inputs[f"b_x{g}"])
        if apply_affine:
            base[f"ga_{g}"] = rep(inputs[f"g_{g}"], np.float32)
            base[f"be_{g}"] = rep(inputs[f"be_{g}"], np.float32)

    hT_full = inputs["h_states"].T  # [H, B]
    xT_full = inputs["inputs"].T
    in_maps = []
    for c in range(NCORES):
        hTc = np.ascontiguousarray(hT_full[:, c * BS : (c + 1) * BS])
        xTc = np.ascontiguousarray(xT_full[:, c * BS : (c + 1) * BS])
        in_maps.append(
            dict(
                base,
                hT=pair_layout(hTc, bf16),
                xT=pair_layout(xTc, bf16),
                hT8=pair_layout(hTc, f8),
                xT8=pair_layout(xTc, f8),
            )
        )

    res = run_bass_kernel_spmd(
        nc,
        in_maps,
        list(range(NCORES)),
        trace=bool(os.environ.get("KERNEL_TRACE")),
    )
    kernel.last_result = res

    c_t = np.concatenate([res.results[c]["c_out"] for c in range(NCORES)], axis=0)
    h_t = np.concatenate([res.results[c]["h_out"] for c in range(NCORES)], axis=0)
    y_t = np.concatenate([res.results[c]["y_out"] for c in range(NCORES)], axis=0)
    return (c_t, h_t, y_t)
